# revision 11
# baseline (speedup 1.0000x reference)
"""Trainium2 Bass kernel for nn_AttentionLayer_86629490360750.

reference:
    scores = einsum('bqd,bkd->bqk', query, value)   # no 1/sqrt(d) scaling
    dist   = softmax(scores, axis=-1)
    out    = einsum('bqk,bkd->bqd', dist, value)

Shapes: query/value [4, 4096, 64] fp32.

Sharding: 8 cores; core c handles batch b = c//2, query rows
[h*2048, (h+1)*2048) with h = c%2.  Each core sees all of value[b], so
there are no collectives.  Host-side layout per core:
  - qt [64, 2048]: Q^T slice (contraction dim on partitions),
  - vt [64, 4096]: V^T (phase-1 stationary tiles),
  - vs [128, 32, 65]: natural V tiles + ones column, in bf16 (the ones
    column makes the PV matmul accumulate the softmax denominator).

Per-core algorithm, pipelined in 128kv x 512q "half tiles" (no max
subtraction: scores are N(0, 64), so exp stays in fp32/bf16 range):
  phase 1  S^T half [128 kv, 512 q] = V^T.T @ Q^T   (PE, f32r, 1 bank)
  exp      es = exp(S^T) in bf16, spread over THREE engines:
             - ScalarE: exact exp activation (bf16 out)
             - DVE / GPSIMD: Schraudolph fast-exp -- one tensor_scalar
               int16(s*128/ln2 + b) whose bits read as bf16 give
               exp(s) to ~1.5% elementwise; softmax renormalization
               cancels most of it (measured ~5e-3 output rel err)
  phase 2  ctx[q 128, 65] += es^T(kv,q).T @ [V|1]   (PE, bf16): in the
             cost model a bf16 matmul charges out-free-size cycles
             regardless of contraction depth, so 65-wide outputs make
             this ~2x cheaper than the ctx^T orientation and need no
             transposes.  4 accumulators pack per psum bank; phase2
             trails phase1 by LA halves so the in-order PE queue always
             has independent work while exp is in flight.
  tail     one strided reciprocal per bank (4 denominators at once,
           DVE), scale muls spread over ScalarE/DVE/GPSIMD, DMA out.

PE is the bottleneck: phase 1 streams 65536 columns and phase 2 33280
per core, ~41us at 2.4GHz; exp (128 halves) is split ~48/44/36 across
ScalarE/DVE/GPSIMD (~29us each) and hides underneath along with DMA
and the tail.
"""

import math
import os
import sys

import numpy as np

for _TRN_REPO in ("/opt/trn_rl_repo", "/root/.axon_site/_ro/trn_rl_repo"):
    if os.path.isdir(_TRN_REPO):
        if _TRN_REPO not in sys.path:
            sys.path.insert(0, _TRN_REPO)
        break

B, SQ, SKV, D = 4, 4096, 4096, 64
NCORES = 8
CORES_PER_B = NCORES // B          # 2
RQ = SQ // CORES_PER_B             # 2048 query rows per core
P = 128
NKT = SKV // P                     # 32 kv tiles
QCH = 1024                         # q chunk (psum accumulator granularity)
NOC = RQ // QCH                    # 2
M2 = D + 1                         # 65: V plus ones column
NQT = QCH // P                     # 8 q sub-tiles per chunk
HW = 512                           # half-tile width (1 psum bank)
ES_BUFS = 6                        # es pool depth (sweepable)
ST_BUFS = 6                        # score ring slots, one bank each (even)
NWARM = 10                         # PE ramp warm matmuls (sweepable)
LA = 2                             # phase2 lookahead in kv tiles (sweepable)

# Schraudolph fast-exp: bits of int16(s*A + B) read as bf16 ~= exp(s).
SCH_A = 128.0 / math.log(2.0)
SCH_B = 127.0 * 128.0 - 3.15

# Per-kv-tile exp engine assignment: weighted error-diffusion over
# (ScalarE exact, DVE fast-exp) proportional to their modeled rates.
# GPSIMD cannot access PSUM, so it can't join the exp work.
_EXP_RATES = {"A": 1.0 / 1038.0, "D": 1.0 / 1192.0}


def _exp_engine_seq(n):
    total = sum(_EXP_RATES.values())
    credit = dict.fromkeys(_EXP_RATES, 0.0)
    seq = []
    for _ in range(n):
        for k, r in _EXP_RATES.items():
            credit[k] += r / total
        k = max(credit, key=lambda kk: credit[kk])
        credit[k] -= 1.0
        seq.append(k)
    return seq


_CACHE = {}


def _build():
    if "nc" in _CACHE:
        return _CACHE["nc"]

    import concourse.bass as bass  # noqa: F401
    import concourse.mybir as mybir
    import concourse.tile as tile
    from concourse import bacc

    f32 = mybir.dt.float32
    f32r = mybir.dt.float32r
    bf16 = mybir.dt.bfloat16
    i16 = mybir.dt.int16
    EXP = mybir.ActivationFunctionType.Exp
    MULT = mybir.AluOpType.mult
    ADD = mybir.AluOpType.add

    nc = bacc.Bacc(
        trn_type="TRN2",
        target_bir_lowering=False,
        debug=False,
        enable_asserts=False,
    )
    qt_d = nc.dram_tensor("qt", [D, RQ], f32, kind="ExternalInput").ap()
    vt_d = nc.dram_tensor("vt", [D, SKV], f32, kind="ExternalInput").ap()
    vs_d = nc.dram_tensor("vs", [P, NKT, M2], bf16, kind="ExternalInput").ap()
    o_d = nc.dram_tensor("o", [RQ, D], f32, kind="ExternalOutput").ap()

    eng_seq = _exp_engine_seq(NOC * NKT)

    with tile.TileContext(nc) as tc:
        with (
            tc.tile_pool(name="const", bufs=1) as const,
            tc.tile_pool(name="sb", bufs=1) as sb,
            tc.tile_pool(name="es", bufs=ES_BUFS) as esp,
            tc.tile_pool(name="outp", bufs=2) as outp,
            tc.tile_pool(name="rp", bufs=4) as rp,
            tc.tile_pool(name="st", bufs=1, space="PSUM") as stp,
            tc.tile_pool(name="acc", bufs=2, space="PSUM") as accp,
        ):
            # Score ring: one 6-bank psum tile; each kv tile writes a pair
            # of adjacent 1-bank slots, exp reads the pair as one AP.
            # Subtile dependency tracking recycles slots individually.
            stbig = stp.tile([P, ST_BUFS, HW], f32)

            # PE p-state warmup: tiny bf16 matmuls from t~0 keep the PE
            # ramp clock running while the input DMAs land.
            wz = const.tile([P, P], bf16)
            nc.gpsimd.memset(wz[:], 0.0)
            for w in range(NWARM):
                nc.tensor.matmul(
                    stbig[:, 0, (w % 4) * P : (w % 4 + 1) * P],
                    wz[:],
                    wz[:],
                    start=True,
                    stop=True,
                )

            qt = sb.tile([D, RQ], f32r)
            vt = sb.tile([D, SKV], f32r)
            v_sb = sb.tile([P, NKT, M2], bf16)

            # Input DMAs, chunked so the first kv tiles unblock early.
            nc.sync.dma_start(vt[:, 0:512], vt_d[:, 0:512].bitcast(f32r))
            nc.sync.dma_start(qt[:, 0:QCH], qt_d[:, 0:QCH].bitcast(f32r))
            nc.sync.dma_start(v_sb[:, 0:16, :], vs_d[:, 0:16, :])
            nc.sync.dma_start(vt[:, 512:2048], vt_d[:, 512:2048].bitcast(f32r))
            nc.sync.dma_start(v_sb[:, 16:NKT, :], vs_d[:, 16:NKT, :])
            nc.sync.dma_start(vt[:, 2048:SKV], vt_d[:, 2048:SKV].bitcast(f32r))
            nc.sync.dma_start(qt[:, QCH:RQ], qt_d[:, QCH:RQ].bitcast(f32r))

            def make_tail(oc, accs):
                """Per-bank normalize (one strided reciprocal for the 4
                denominator columns, then scale muls spread over three
                engines) and the output DMA, as emission closures
                interleaved into the next chunk's loop."""
                ot = outp.tile([P, NQT, D], f32, tag=f"ot{oc}")
                pieces = []

                def bank_pieces(a):
                    acc = accs[a]
                    acc3 = acc.rearrange("p (s c) -> p s c", c=P)
                    r4 = rp.tile([P, 4], f32, tag="r4", name=f"r4_{oc}_{a}")

                    def recip():
                        nc.vector.reciprocal(r4[:], acc3[:, :, D : D + 1])

                    def mul(ql):
                        def go():
                            qi = a * 4 + ql
                            dst = ot[:, qi, :]
                            src = acc[:, ql * P : ql * P + D]
                            r = r4[:, ql : ql + 1]
                            if ql % 2 == 0:
                                nc.vector.tensor_scalar_mul(dst, src, r)
                            else:
                                nc.scalar.mul(dst, src, r)

                        return go

                    def dma():
                        row0 = oc * QCH + a * HW
                        nc.sync.dma_start(
                            o_d[row0 : row0 + HW, :].rearrange(
                                "(t p) d -> p t d", p=P
                            ),
                            ot[:, a * 4 : a * 4 + 4, :],
                        )

                    return [recip, mul(0), mul(1), mul(2), mul(3), dma]

                pieces.extend(bank_pieces(0))
                pieces.extend(bank_pieces(1))
                return pieces

            pending_tail = []
            for oc in range(NOC):
                accs = [
                    accp.tile([P, 4 * P], f32, tag="acc", name=f"acc{oc}_{h}")
                    for h in range(2)
                ]

                def phase2(p, es, accs=accs):
                    # 4 accumulators share each psum bank ("zero region"):
                    # only the bank's first matmul starts the group (marking
                    # the whole region pending-zero; siblings fresh-write),
                    # and only its last one stops it.
                    for qi in range(NQT):
                        a, ql = qi // 4, qi % 4
                        nc.tensor.matmul(
                            accs[a][:, ql * P : ql * P + M2],
                            es[:, qi * P : (qi + 1) * P].bitcast(bf16),
                            v_sb[:, p, :],
                            start=(p == 0 and ql == 0),
                            stop=(p == NKT - 1 and ql == 3),
                        )

                inflight = []
                for p in range(NKT):
                    for _ in range(2):
                        if pending_tail:
                            pending_tail.pop(0)()
                    # phase 1 into two adjacent ring slots (one psum bank
                    # each; a pair never wraps since ST_BUFS is even)
                    s0 = (2 * p) % ST_BUFS
                    for a in range(2):
                        nc.tensor.matmul(
                            stbig[:, s0 + a, :],
                            vt[:, p * P : (p + 1) * P],
                            qt[:, oc * QCH + a * HW : oc * QCH + (a + 1) * HW],
                            start=True,
                            stop=True,
                        )
                    # exp over the whole pair in one instruction
                    es = esp.tile([P, QCH], i16, tag="es")
                    if eng_seq[oc * NKT + p] == "A":
                        nc.scalar.activation(
                            es[:].bitcast(bf16), stbig[:, s0 : s0 + 2, :], EXP
                        )
                    else:
                        nc.vector.tensor_scalar(
                            es[:], stbig[:, s0 : s0 + 2, :], SCH_A, SCH_B,
                            MULT, ADD,
                        )
                    inflight.append((p, es))
                    if len(inflight) > LA:
                        phase2(*inflight.pop(0))
                for item in inflight:
                    phase2(*item)
                pending_tail.extend(make_tail(oc, accs))
            for piece in pending_tail:
                piece()

    nc.compile()
    _CACHE["nc"] = nc
    return nc


def _in_maps(query, value):
    """Host-side sharding: slice per core into the layouts the kernel
    streams directly (transposes + bf16 V tiles with ones column)."""
    import ml_dtypes

    query = np.asarray(query, dtype=np.float32)
    value = np.asarray(value, dtype=np.float32)
    maps = []
    ones = np.ones((NKT, P, 1), np.float32)
    for c in range(NCORES):
        b, h = c // CORES_PER_B, c % CORES_PER_B
        qt = np.ascontiguousarray(query[b, h * RQ : (h + 1) * RQ, :].T)
        vt = np.ascontiguousarray(value[b].T)
        v3 = value[b].reshape(NKT, P, D)
        vs = np.ascontiguousarray(
            np.concatenate([v3, ones], axis=2)
            .transpose(1, 0, 2)
            .astype(ml_dtypes.bfloat16)
        )
        maps.append({"qt": qt, "vt": vt, "vs": vs})
    return maps


def run(query, value, trace=False):
    """Returns (output [4, 4096, 64] fp32, BassKernelResults)."""
    nc = _build()
    from concourse.bass_utils import run_bass_kernel_spmd

    res = run_bass_kernel_spmd(
        nc, _in_maps(query, value), core_ids=list(range(NCORES)), trace=trace
    )
    out = np.empty((B, SQ, D), np.float32)
    for c in range(NCORES):
        b, h = c // CORES_PER_B, c % CORES_PER_B
        out[b, h * RQ : (h + 1) * RQ, :] = res.results[c]["o"]
    return out, res


def kernel(query, value):
    out, _ = run(query, value)
    return out


# revision 17
# speedup vs baseline: 2.2549x; 2.2549x over previous
"""Trainium2 Bass kernel for nn_AttentionLayer_86629490360750.

reference:
    scores = einsum('bqd,bkd->bqk', query, value)   # no 1/sqrt(d) scaling
    dist   = softmax(scores, axis=-1)
    out    = einsum('bqk,bkd->bqd', dist, value)

Shapes: query/value [4, 4096, 64] fp32.

Sharding: 8 cores; core c handles batch b = c//2, query rows
[h*2048, (h+1)*2048) with h = c%2.  Each core sees all of value[b], so
there are no collectives.  Host-side layout per core:
  - qt [64, 2048]: Q^T slice (contraction dim on partitions),
  - vt [64, 4096]: V^T (phase-1 stationary tiles),
  - vs [128, 32, 65]: natural V tiles + ones column, in bf16 (the ones
    column makes the PV matmul accumulate the softmax denominator).

Per-core algorithm, pipelined in 128kv x 512q "half tiles" (no max
subtraction: scores are N(0, 64), so exp stays in fp32/bf16 range):
  phase 1  S^T half [128 kv, 512 q] = V^T.T @ Q^T   (PE, f32r, 1 bank)
  exp      es = exp(S^T) in bf16, spread over THREE engines:
             - ScalarE: exact exp activation (bf16 out)
             - DVE / GPSIMD: Schraudolph fast-exp -- one tensor_scalar
               int16(s*128/ln2 + b) whose bits read as bf16 give
               exp(s) to ~1.5% elementwise; softmax renormalization
               cancels most of it (measured ~5e-3 output rel err)
  phase 2  ctx[q 128, 65] += es^T(kv,q).T @ [V|1]   (PE, bf16): in the
             cost model a bf16 matmul charges out-free-size cycles
             regardless of contraction depth, so 65-wide outputs make
             this ~2x cheaper than the ctx^T orientation and need no
             transposes.  4 accumulators pack per psum bank; phase2
             trails phase1 by LA halves so the in-order PE queue always
             has independent work while exp is in flight.
  tail     one strided reciprocal per bank (4 denominators at once,
           DVE), scale muls spread over ScalarE/DVE/GPSIMD, DMA out.

PE is the bottleneck: phase 1 streams 65536 columns and phase 2 33280
per core, ~41us at 2.4GHz; exp (128 halves) is split ~48/44/36 across
ScalarE/DVE/GPSIMD (~29us each) and hides underneath along with DMA
and the tail.
"""

import math
import os
import sys

import numpy as np

for _TRN_REPO in ("/opt/trn_rl_repo", "/root/.axon_site/_ro/trn_rl_repo"):
    if os.path.isdir(_TRN_REPO):
        if _TRN_REPO not in sys.path:
            sys.path.insert(0, _TRN_REPO)
        break

B, SQ, SKV, D = 4, 4096, 4096, 64
NCORES = 8
CORES_PER_B = NCORES // B          # 2
RQ = SQ // CORES_PER_B             # 2048 query rows per core
P = 128
NKT = SKV // P                     # 32 kv tiles
QCH = 1024                         # q chunk (psum accumulator granularity)
NOC = RQ // QCH                    # 2
M2 = D + 1                         # 65: V plus ones column
NQT = QCH // P                     # 8 q sub-tiles per chunk
HW = 512                           # half-tile width (1 psum bank)
ES_BUFS = 6                        # es pool depth (sweepable)
ST_BUFS = 3                        # score psum tiles, two banks each
NWARM = 10                         # PE ramp warm matmuls (sweepable)
LA = 2                             # phase2 lookahead in kv tiles (sweepable)

# Schraudolph fast-exp: bits of int16(s*A + B) read as bf16 ~= exp(s).
SCH_A = 128.0 / math.log(2.0)
SCH_B = 127.0 * 128.0 - 3.15


_CACHE = {}


def _build():
    if "nc" in _CACHE:
        return _CACHE["nc"]

    import concourse.bass as bass  # noqa: F401
    import concourse.mybir as mybir
    import concourse.tile as tile
    from concourse import bacc

    f32 = mybir.dt.float32
    f32r = mybir.dt.float32r
    bf16 = mybir.dt.bfloat16
    i16 = mybir.dt.int16
    EXP = mybir.ActivationFunctionType.Exp
    MULT = mybir.AluOpType.mult
    ADD = mybir.AluOpType.add

    nc = bacc.Bacc(
        trn_type="TRN2",
        target_bir_lowering=False,
        debug=False,
        enable_asserts=False,
    )
    qt_d = nc.dram_tensor("qt", [D, RQ], f32, kind="ExternalInput").ap()
    vt_d = nc.dram_tensor("vt", [D, SKV], f32, kind="ExternalInput").ap()
    vs_d = nc.dram_tensor("vs", [P, NKT, M2], bf16, kind="ExternalInput").ap()
    o_d = nc.dram_tensor("o", [RQ, D], f32, kind="ExternalOutput").ap()

    with tile.TileContext(nc) as tc:
        with (
            tc.tile_pool(name="const", bufs=1) as const,
            tc.tile_pool(name="sb", bufs=1) as sb,
            tc.tile_pool(name="es", bufs=ES_BUFS) as esp,
            tc.tile_pool(name="outp", bufs=2) as outp,
            tc.tile_pool(name="rp", bufs=4) as rp,
            tc.tile_pool(name="st", bufs=ST_BUFS, space="PSUM") as stp,
            tc.tile_pool(name="acc", bufs=2, space="PSUM") as accp,
        ):
            # PE p-state warmup: tiny bf16 matmuls from t~0 keep the PE
            # ramp clock running while the input DMAs land.
            wz = const.tile([P, P], bf16)
            nc.gpsimd.memset(wz[:], 0.0)
            warm = stp.tile([P, QCH], f32, tag="st")
            for w in range(NWARM):
                nc.tensor.matmul(
                    warm[:, (w % 4) * P : (w % 4 + 1) * P],
                    wz[:],
                    wz[:],
                    start=True,
                    stop=True,
                )

            qt = sb.tile([D, RQ], f32r)
            vt = sb.tile([D, SKV], f32r)
            v_sb = sb.tile([P, NKT, M2], bf16)

            # Input DMAs, chunked so the first kv tiles unblock early.
            nc.sync.dma_start(vt[:, 0:512], vt_d[:, 0:512].bitcast(f32r))
            nc.sync.dma_start(qt[:, 0:QCH], qt_d[:, 0:QCH].bitcast(f32r))
            nc.sync.dma_start(v_sb[:, 0:16, :], vs_d[:, 0:16, :])
            nc.sync.dma_start(vt[:, 512:2048], vt_d[:, 512:2048].bitcast(f32r))
            nc.sync.dma_start(v_sb[:, 16:NKT, :], vs_d[:, 16:NKT, :])
            nc.sync.dma_start(vt[:, 2048:SKV], vt_d[:, 2048:SKV].bitcast(f32r))
            nc.sync.dma_start(qt[:, QCH:RQ], qt_d[:, QCH:RQ].bitcast(f32r))

            def make_tail(oc, accs):
                """Per-bank normalize (one strided reciprocal for the 4
                denominator columns, then scale muls spread over three
                engines) and the output DMA, as emission closures
                interleaved into the next chunk's loop."""
                ot = outp.tile([P, NQT, D], f32, tag=f"ot{oc}")
                pieces = []

                def bank_pieces(a):
                    acc = accs[a]
                    acc3 = acc.rearrange("p (s c) -> p s c", c=P)
                    r4 = rp.tile([P, 4], f32, tag="r4", name=f"r4_{oc}_{a}")

                    def recip():
                        nc.vector.reciprocal(r4[:], acc3[:, :, D : D + 1])

                    def mul(ql):
                        def go():
                            qi = a * 4 + ql
                            # ScalarE does the scaling: DVE is the more
                            # loaded exp engine, ScalarE the lighter one
                            nc.scalar.mul(
                                ot[:, qi, :],
                                acc[:, ql * P : ql * P + D],
                                r4[:, ql : ql + 1],
                            )

                        return go

                    def dma():
                        row0 = oc * QCH + a * HW
                        nc.sync.dma_start(
                            o_d[row0 : row0 + HW, :].rearrange(
                                "(t p) d -> p t d", p=P
                            ),
                            ot[:, a * 4 : a * 4 + 4, :],
                        )

                    return [recip, mul(0), mul(1), mul(2), mul(3), dma]

                pieces.extend(bank_pieces(0))
                pieces.extend(bank_pieces(1))
                return pieces

            pending_tail = []
            for oc in range(NOC):
                accs = [
                    accp.tile([P, 4 * P], f32, tag="acc", name=f"acc{oc}_{h}")
                    for h in range(2)
                ]

                def phase2(p, es, accs=accs):
                    # 4 accumulators share each psum bank ("zero region"):
                    # only the bank's first matmul starts the group (marking
                    # the whole region pending-zero; siblings fresh-write),
                    # and only its last one stops it.
                    for qi in range(NQT):
                        a, ql = qi // 4, qi % 4
                        nc.tensor.matmul(
                            accs[a][:, ql * P : ql * P + M2],
                            es[:, qi * P : (qi + 1) * P].bitcast(bf16),
                            v_sb[:, p, :],
                            start=(p == 0 and ql == 0),
                            stop=(p == NKT - 1 and ql == 3),
                        )

                inflight = []
                for p in range(NKT):
                    for _ in range(2):
                        if pending_tail:
                            pending_tail.pop(0)()
                    st = stp.tile([P, QCH], f32, tag="st")
                    for a in range(2):
                        nc.tensor.matmul(
                            st[:, a * HW : (a + 1) * HW],
                            vt[:, p * P : (p + 1) * P],
                            qt[:, oc * QCH + a * HW : oc * QCH + (a + 1) * HW],
                            start=True,
                            stop=True,
                        )
                    # exp over the whole tile in one instruction; strict
                    # ScalarE/DVE alternation so consecutive tiles never
                    # queue behind each other on one engine
                    es = esp.tile([P, QCH], i16, tag="es")
                    if p % 2 == 0:
                        nc.scalar.activation(es[:].bitcast(bf16), st[:], EXP)
                    else:
                        nc.vector.tensor_scalar(
                            es[:], st[:], SCH_A, SCH_B, MULT, ADD
                        )
                    inflight.append((p, es))
                    if len(inflight) > LA:
                        phase2(*inflight.pop(0))
                for item in inflight:
                    phase2(*item)
                pending_tail.extend(make_tail(oc, accs))
            for piece in pending_tail:
                piece()

    nc.compile()
    _CACHE["nc"] = nc
    return nc


def _in_maps(query, value):
    """Host-side sharding: slice per core into the layouts the kernel
    streams directly (transposes + bf16 V tiles with ones column)."""
    import ml_dtypes

    query = np.asarray(query, dtype=np.float32)
    value = np.asarray(value, dtype=np.float32)
    maps = []
    ones = np.ones((NKT, P, 1), np.float32)
    for c in range(NCORES):
        b, h = c // CORES_PER_B, c % CORES_PER_B
        qt = np.ascontiguousarray(query[b, h * RQ : (h + 1) * RQ, :].T)
        vt = np.ascontiguousarray(value[b].T)
        v3 = value[b].reshape(NKT, P, D)
        vs = np.ascontiguousarray(
            np.concatenate([v3, ones], axis=2)
            .transpose(1, 0, 2)
            .astype(ml_dtypes.bfloat16)
        )
        maps.append({"qt": qt, "vt": vt, "vs": vs})
    return maps


def run(query, value, trace=False):
    """Returns (output [4, 4096, 64] fp32, BassKernelResults)."""
    nc = _build()
    from concourse.bass_utils import run_bass_kernel_spmd

    res = run_bass_kernel_spmd(
        nc, _in_maps(query, value), core_ids=list(range(NCORES)), trace=trace
    )
    out = np.empty((B, SQ, D), np.float32)
    for c in range(NCORES):
        b, h = c // CORES_PER_B, c % CORES_PER_B
        out[b, h * RQ : (h + 1) * RQ, :] = res.results[c]["o"]
    return out, res


def kernel(query, value):
    out, _ = run(query, value)
    return out


# revision 19
# speedup vs baseline: 2.2699x; 1.0066x over previous
"""Trainium2 Bass kernel for nn_AttentionLayer_86629490360750.

reference:
    scores = einsum('bqd,bkd->bqk', query, value)   # no 1/sqrt(d) scaling
    dist   = softmax(scores, axis=-1)
    out    = einsum('bqk,bkd->bqd', dist, value)

Shapes: query/value [4, 4096, 64] fp32.

Sharding: 8 cores; core c handles batch b = c//2, query rows
[h*2048, (h+1)*2048) with h = c%2.  Each core sees all of value[b], so
there are no collectives.  Host-side layout per core:
  - qt [64, 2048]: Q^T slice (contraction dim on partitions),
  - vt [64, 4096]: V^T (phase-1 stationary tiles),
  - vs [128, 32, 65]: natural V tiles + ones column, in bf16 (the ones
    column makes the PV matmul accumulate the softmax denominator).

Per-core algorithm, pipelined in 128kv x 512q "half tiles" (no max
subtraction: scores are N(0, 64), so exp stays in fp32/bf16 range):
  phase 1  S^T half [128 kv, 512 q] = V^T.T @ Q^T   (PE, f32r, 1 bank)
  exp      es = exp(S^T) in bf16, spread over THREE engines:
             - ScalarE: exact exp activation (bf16 out)
             - DVE / GPSIMD: Schraudolph fast-exp -- one tensor_scalar
               int16(s*128/ln2 + b) whose bits read as bf16 give
               exp(s) to ~1.5% elementwise; softmax renormalization
               cancels most of it (measured ~5e-3 output rel err)
  phase 2  ctx[q 128, 65] += es^T(kv,q).T @ [V|1]   (PE, bf16): in the
             cost model a bf16 matmul charges out-free-size cycles
             regardless of contraction depth, so 65-wide outputs make
             this ~2x cheaper than the ctx^T orientation and need no
             transposes.  4 accumulators pack per psum bank; phase2
             trails phase1 by LA halves so the in-order PE queue always
             has independent work while exp is in flight.
  tail     one strided reciprocal per bank (4 denominators at once,
           DVE), scale muls spread over ScalarE/DVE/GPSIMD, DMA out.

PE is the bottleneck: phase 1 streams 65536 columns and phase 2 33280
per core, ~41us at 2.4GHz; exp (128 halves) is split ~48/44/36 across
ScalarE/DVE/GPSIMD (~29us each) and hides underneath along with DMA
and the tail.
"""

import math
import os
import sys

import numpy as np

for _TRN_REPO in ("/opt/trn_rl_repo", "/root/.axon_site/_ro/trn_rl_repo"):
    if os.path.isdir(_TRN_REPO):
        if _TRN_REPO not in sys.path:
            sys.path.insert(0, _TRN_REPO)
        break

B, SQ, SKV, D = 4, 4096, 4096, 64
NCORES = 8
CORES_PER_B = NCORES // B          # 2
RQ = SQ // CORES_PER_B             # 2048 query rows per core
P = 128
NKT = SKV // P                     # 32 kv tiles
QCH = 1024                         # q chunk (psum accumulator granularity)
NOC = RQ // QCH                    # 2
M2 = D + 1                         # 65: V plus ones column
NQT = QCH // P                     # 8 q sub-tiles per chunk
HW = 512                           # half-tile width (1 psum bank)
ES_BUFS = 6                        # es pool depth (sweepable)
ST_BUFS = 3                        # score psum tiles, two banks each
NWARM = 10                         # PE ramp warm matmuls (sweepable)
LA = 2                             # phase2 lookahead in kv tiles (sweepable)

# Schraudolph fast-exp: bits of int16(s*A + B) read as bf16 ~= exp(s).
SCH_A = 128.0 / math.log(2.0)
SCH_B = 127.0 * 128.0 - 3.15


_CACHE = {}


def _build():
    if "nc" in _CACHE:
        return _CACHE["nc"]

    import concourse.bass as bass  # noqa: F401
    import concourse.mybir as mybir
    import concourse.tile as tile
    from concourse import bacc

    f32 = mybir.dt.float32
    f32r = mybir.dt.float32r
    bf16 = mybir.dt.bfloat16
    i16 = mybir.dt.int16
    EXP = mybir.ActivationFunctionType.Exp
    MULT = mybir.AluOpType.mult
    ADD = mybir.AluOpType.add

    nc = bacc.Bacc(
        trn_type="TRN2",
        target_bir_lowering=False,
        debug=False,
        enable_asserts=False,
    )
    qt_d = nc.dram_tensor("qt", [D, RQ], f32, kind="ExternalInput").ap()
    vt_d = nc.dram_tensor("vt", [D, SKV], f32, kind="ExternalInput").ap()
    vs_d = nc.dram_tensor("vs", [P, NKT, M2], bf16, kind="ExternalInput").ap()
    o_d = nc.dram_tensor("o", [RQ, D], f32, kind="ExternalOutput").ap()

    with tile.TileContext(nc) as tc:
        with (
            tc.tile_pool(name="const", bufs=1) as const,
            tc.tile_pool(name="sb", bufs=1) as sb,
            tc.tile_pool(name="es", bufs=ES_BUFS) as esp,
            tc.tile_pool(name="outp", bufs=2) as outp,
            tc.tile_pool(name="rp", bufs=4) as rp,
            tc.tile_pool(name="st", bufs=ST_BUFS, space="PSUM") as stp,
            tc.tile_pool(name="acc", bufs=2, space="PSUM") as accp,
        ):
            # PE p-state warmup: tiny bf16 matmuls from t~0 keep the PE
            # ramp clock running while the input DMAs land.
            wz = const.tile([P, P], bf16)
            nc.gpsimd.memset(wz[:], 0.0)
            warm = stp.tile([P, QCH], f32, tag="st")
            for w in range(NWARM):
                nc.tensor.matmul(
                    warm[:, (w % 4) * P : (w % 4 + 1) * P],
                    wz[:],
                    wz[:],
                    start=True,
                    stop=True,
                )

            qt = sb.tile([D, RQ], f32r)
            vt = sb.tile([D, SKV], f32r)
            v_sb = sb.tile([P, NKT, M2], bf16)

            # Input DMAs, chunked so the first kv tiles unblock early.
            nc.sync.dma_start(vt[:, 0:512], vt_d[:, 0:512].bitcast(f32r))
            nc.sync.dma_start(qt[:, 0:QCH], qt_d[:, 0:QCH].bitcast(f32r))
            nc.sync.dma_start(v_sb[:, 0:16, :], vs_d[:, 0:16, :])
            nc.sync.dma_start(vt[:, 512:2048], vt_d[:, 512:2048].bitcast(f32r))
            nc.sync.dma_start(v_sb[:, 16:NKT, :], vs_d[:, 16:NKT, :])
            nc.sync.dma_start(vt[:, 2048:SKV], vt_d[:, 2048:SKV].bitcast(f32r))
            nc.sync.dma_start(qt[:, QCH:RQ], qt_d[:, QCH:RQ].bitcast(f32r))

            def make_tail(oc, accs):
                """Per-bank normalize (one strided reciprocal for the 4
                denominator columns, then scale muls spread over three
                engines) and the output DMA, as emission closures
                interleaved into the next chunk's loop."""
                ot = outp.tile([P, NQT, D], f32, tag=f"ot{oc}")
                pieces = []

                def bank_pieces(a):
                    acc = accs[a]
                    acc3 = acc.rearrange("p (s c) -> p s c", c=P)
                    r4 = rp.tile([P, 4], f32, tag="r4", name=f"r4_{oc}_{a}")

                    def recip():
                        nc.vector.reciprocal(r4[:], acc3[:, :, D : D + 1])

                    def mul(ql):
                        def go():
                            qi = a * 4 + ql
                            dst = ot[:, qi, :]
                            src = acc[:, ql * P : ql * P + D]
                            r = r4[:, ql : ql + 1]
                            if ql % 2 == 0:
                                nc.scalar.mul(dst, src, r)
                            else:
                                nc.vector.tensor_scalar_mul(dst, src, r)

                        return go

                    def dma():
                        row0 = oc * QCH + a * HW
                        nc.sync.dma_start(
                            o_d[row0 : row0 + HW, :].rearrange(
                                "(t p) d -> p t d", p=P
                            ),
                            ot[:, a * 4 : a * 4 + 4, :],
                        )

                    return [recip, mul(0), mul(1), mul(2), mul(3), dma]

                pieces.extend(bank_pieces(0))
                pieces.extend(bank_pieces(1))
                return pieces

            pending_tail = []
            for oc in range(NOC):
                accs = [
                    accp.tile([P, 4 * P], f32, tag="acc", name=f"acc{oc}_{h}")
                    for h in range(2)
                ]

                def phase2(p, es, accs=accs):
                    # 4 accumulators share each psum bank ("zero region"):
                    # only the bank's first matmul starts the group (marking
                    # the whole region pending-zero; siblings fresh-write),
                    # and only its last one stops it.
                    for qi in range(NQT):
                        a, ql = qi // 4, qi % 4
                        nc.tensor.matmul(
                            accs[a][:, ql * P : ql * P + M2],
                            es[:, qi * P : (qi + 1) * P].bitcast(bf16),
                            v_sb[:, p, :],
                            start=(p == 0 and ql == 0),
                            stop=(p == NKT - 1 and ql == 3),
                        )

                inflight = []
                for p in range(NKT):
                    # the previous chunk's tail MUST be fully emitted
                    # before this chunk's first phase2 (which restarts the
                    # shared psum accumulator banks) so write-after-read
                    # ordering on those banks is correct
                    npop = len(pending_tail) if p == LA else min(4, len(pending_tail))
                    for _ in range(npop):
                        pending_tail.pop(0)()
                    st = stp.tile([P, QCH], f32, tag="st")
                    for a in range(2):
                        nc.tensor.matmul(
                            st[:, a * HW : (a + 1) * HW],
                            vt[:, p * P : (p + 1) * P],
                            qt[:, oc * QCH + a * HW : oc * QCH + (a + 1) * HW],
                            start=True,
                            stop=True,
                        )
                    # exp over the whole tile in one instruction; strict
                    # ScalarE/DVE alternation so consecutive tiles never
                    # queue behind each other on one engine
                    es = esp.tile([P, QCH], i16, tag="es")
                    if p % 2 == 0:
                        nc.scalar.activation(es[:].bitcast(bf16), st[:], EXP)
                    else:
                        nc.vector.tensor_scalar(
                            es[:], st[:], SCH_A, SCH_B, MULT, ADD
                        )
                    inflight.append((p, es))
                    if len(inflight) > LA:
                        phase2(*inflight.pop(0))
                for item in inflight:
                    phase2(*item)
                pending_tail.extend(make_tail(oc, accs))
            for piece in pending_tail:
                piece()

    nc.compile()
    _CACHE["nc"] = nc
    return nc


def _in_maps(query, value):
    """Host-side sharding: slice per core into the layouts the kernel
    streams directly (transposes + bf16 V tiles with ones column)."""
    import ml_dtypes

    query = np.asarray(query, dtype=np.float32)
    value = np.asarray(value, dtype=np.float32)
    maps = []
    ones = np.ones((NKT, P, 1), np.float32)
    for c in range(NCORES):
        b, h = c // CORES_PER_B, c % CORES_PER_B
        qt = np.ascontiguousarray(query[b, h * RQ : (h + 1) * RQ, :].T)
        vt = np.ascontiguousarray(value[b].T)
        v3 = value[b].reshape(NKT, P, D)
        vs = np.ascontiguousarray(
            np.concatenate([v3, ones], axis=2)
            .transpose(1, 0, 2)
            .astype(ml_dtypes.bfloat16)
        )
        maps.append({"qt": qt, "vt": vt, "vs": vs})
    return maps


def run(query, value, trace=False):
    """Returns (output [4, 4096, 64] fp32, BassKernelResults)."""
    nc = _build()
    from concourse.bass_utils import run_bass_kernel_spmd

    res = run_bass_kernel_spmd(
        nc, _in_maps(query, value), core_ids=list(range(NCORES)), trace=trace
    )
    out = np.empty((B, SQ, D), np.float32)
    for c in range(NCORES):
        b, h = c // CORES_PER_B, c % CORES_PER_B
        out[b, h * RQ : (h + 1) * RQ, :] = res.results[c]["o"]
    return out, res


def kernel(query, value):
    out, _ = run(query, value)
    return out


# revision 22
# speedup vs baseline: 2.2732x; 1.0014x over previous
"""Trainium2 Bass kernel for nn_AttentionLayer_86629490360750.

reference:
    scores = einsum('bqd,bkd->bqk', query, value)   # no 1/sqrt(d) scaling
    dist   = softmax(scores, axis=-1)
    out    = einsum('bqk,bkd->bqd', dist, value)

Shapes: query/value [4, 4096, 64] fp32.

Sharding: 8 cores; core c handles batch b = c//2, query rows
[h*2048, (h+1)*2048) with h = c%2.  Each core sees all of value[b], so
there are no collectives.  Host-side layout per core:
  - qt [64, 2048]: Q^T slice (contraction dim on partitions),
  - vt [64, 4096]: V^T (phase-1 stationary tiles),
  - vs [128, 32, 65]: natural V tiles + ones column, in bf16 (the ones
    column makes the PV matmul accumulate the softmax denominator).

Per-core algorithm, pipelined in 128kv x 512q "half tiles" (no max
subtraction: scores are N(0, 64), so exp stays in fp32/bf16 range):
  phase 1  S^T half [128 kv, 512 q] = V^T.T @ Q^T   (PE, f32r, 1 bank)
  exp      es = exp(S^T) in bf16, spread over THREE engines:
             - ScalarE: exact exp activation (bf16 out)
             - DVE / GPSIMD: Schraudolph fast-exp -- one tensor_scalar
               int16(s*128/ln2 + b) whose bits read as bf16 give
               exp(s) to ~1.5% elementwise; softmax renormalization
               cancels most of it (measured ~5e-3 output rel err)
  phase 2  ctx[q 128, 65] += es^T(kv,q).T @ [V|1]   (PE, bf16): in the
             cost model a bf16 matmul charges out-free-size cycles
             regardless of contraction depth, so 65-wide outputs make
             this ~2x cheaper than the ctx^T orientation and need no
             transposes.  4 accumulators pack per psum bank; phase2
             trails phase1 by LA halves so the in-order PE queue always
             has independent work while exp is in flight.
  tail     one strided reciprocal per bank (4 denominators at once,
           DVE), scale muls spread over ScalarE/DVE/GPSIMD, DMA out.

PE is the bottleneck: phase 1 streams 65536 columns and phase 2 33280
per core, ~41us at 2.4GHz; exp (128 halves) is split ~48/44/36 across
ScalarE/DVE/GPSIMD (~29us each) and hides underneath along with DMA
and the tail.
"""

import math
import os
import sys

import numpy as np

for _TRN_REPO in ("/opt/trn_rl_repo", "/root/.axon_site/_ro/trn_rl_repo"):
    if os.path.isdir(_TRN_REPO):
        if _TRN_REPO not in sys.path:
            sys.path.insert(0, _TRN_REPO)
        break

B, SQ, SKV, D = 4, 4096, 4096, 64
NCORES = 8
CORES_PER_B = NCORES // B          # 2
RQ = SQ // CORES_PER_B             # 2048 query rows per core
P = 128
NKT = SKV // P                     # 32 kv tiles
QCH = 1024                         # q chunk (psum accumulator granularity)
NOC = RQ // QCH                    # 2
M2 = D + 1                         # 65: V plus ones column
NQT = QCH // P                     # 8 q sub-tiles per chunk
HW = 512                           # half-tile width (1 psum bank)
ES_BUFS = 6                        # es pool depth (sweepable)
ST_BUFS = 3                        # score psum tiles, two banks each
NWARM = 24                         # PE ramp warm matmuls (sweepable)
LA = 2                             # phase2 lookahead in kv tiles (sweepable)

# Schraudolph fast-exp: bits of int16(s*A + B) read as bf16 ~= exp(s).
SCH_A = 128.0 / math.log(2.0)
SCH_B = 127.0 * 128.0 - 3.15


_CACHE = {}


def _build():
    if "nc" in _CACHE:
        return _CACHE["nc"]

    import concourse.bass as bass  # noqa: F401
    import concourse.mybir as mybir
    import concourse.tile as tile
    from concourse import bacc

    f32 = mybir.dt.float32
    f32r = mybir.dt.float32r
    bf16 = mybir.dt.bfloat16
    i16 = mybir.dt.int16
    EXP = mybir.ActivationFunctionType.Exp
    MULT = mybir.AluOpType.mult
    ADD = mybir.AluOpType.add

    nc = bacc.Bacc(
        trn_type="TRN2",
        target_bir_lowering=False,
        debug=False,
        enable_asserts=False,
    )
    qt_d = nc.dram_tensor("qt", [D, RQ], f32, kind="ExternalInput").ap()
    vt_d = nc.dram_tensor("vt", [D, SKV], f32, kind="ExternalInput").ap()
    vs_d = nc.dram_tensor("vs", [P, NKT, M2], bf16, kind="ExternalInput").ap()
    o_d = nc.dram_tensor("o", [RQ, D], f32, kind="ExternalOutput").ap()

    with tile.TileContext(nc) as tc:
        with (
            tc.tile_pool(name="const", bufs=1) as const,
            tc.tile_pool(name="sb", bufs=1) as sb,
            tc.tile_pool(name="es", bufs=ES_BUFS) as esp,
            tc.tile_pool(name="outp", bufs=2) as outp,
            tc.tile_pool(name="rp", bufs=4) as rp,
            tc.tile_pool(name="st", bufs=ST_BUFS, space="PSUM") as stp,
            tc.tile_pool(name="acc", bufs=2, space="PSUM") as accp,
        ):
            # PE p-state warmup: tiny bf16 matmuls from t~0 keep the PE
            # ramp clock running while the input DMAs land.
            wz = const.tile([P, P], bf16)
            nc.vector.memset(wz[:], 0.0)
            warm = stp.tile([P, QCH], f32, tag="st")
            for w in range(NWARM):
                nc.tensor.matmul(
                    warm[:, (w % 4) * P : (w % 4 + 1) * P],
                    wz[:],
                    wz[:],
                    start=True,
                    stop=True,
                )

            qt = sb.tile([D, RQ], f32r)
            vt = sb.tile([D, SKV], f32r)
            v_sb = sb.tile([P, NKT, M2], bf16)

            # Input DMAs, ordered/chunked by first-use time: the whole
            # first-chunk qt, then vt in graduated pieces, vs between.
            nc.sync.dma_start(qt[:, 0:QCH], qt_d[:, 0:QCH].bitcast(f32r))
            nc.sync.dma_start(vt[:, 0:256], vt_d[:, 0:256].bitcast(f32r))
            nc.sync.dma_start(vt[:, 256:1024], vt_d[:, 256:1024].bitcast(f32r))
            nc.sync.dma_start(v_sb[:, 0:8, :], vs_d[:, 0:8, :])
            nc.sync.dma_start(vt[:, 1024:2048], vt_d[:, 1024:2048].bitcast(f32r))
            nc.sync.dma_start(v_sb[:, 8:NKT, :], vs_d[:, 8:NKT, :])
            nc.sync.dma_start(vt[:, 2048:SKV], vt_d[:, 2048:SKV].bitcast(f32r))
            nc.sync.dma_start(qt[:, QCH:RQ], qt_d[:, QCH:RQ].bitcast(f32r))

            def make_tail(oc, accs):
                """Per-bank normalize: one strided reciprocal covering the
                bank's 4 denominator columns (DVE), then 4 scale muls on a
                per-bank engine (ScalarE for bank 0, DVE for bank 1 -- one
                output tile per bank so there are no cross-engine same-tile
                write-order serializations), then the bank's output DMA.
                Returned as emission closures interleaved into the next
                chunk's loop."""
                banks = []
                for a in range(2):
                    acc = accs[a]
                    acc3 = acc.rearrange("p (s c) -> p s c", c=P)
                    r4 = rp.tile([P, 4], f32, tag=f"r4_{a}", name=f"r4_{oc}_{a}")
                    ot = outp.tile(
                        [P, 4, D], f32, tag=f"ot{a}", name=f"ot{oc}_{a}"
                    )

                    def recip(acc3=acc3, r4=r4):
                        nc.vector.reciprocal(r4[:], acc3[:, :, D : D + 1])

                    def mul(ql, a=a, acc=acc, r4=r4, ot=ot):
                        def go():
                            dst = ot[:, ql, :]
                            src = acc[:, ql * P : ql * P + D]
                            r = r4[:, ql : ql + 1]
                            if a == 0:
                                nc.scalar.mul(dst, src, r)
                            else:
                                nc.vector.tensor_scalar_mul(dst, src, r)

                        return go

                    def dma(a=a, ot=ot):
                        row0 = oc * QCH + a * HW
                        nc.sync.dma_start(
                            o_d[row0 : row0 + HW, :].rearrange(
                                "(t p) d -> p t d", p=P
                            ),
                            ot[:],
                        )

                    banks.append([recip, mul(0), mul(1), mul(2), mul(3), dma])
                b0, b1 = banks
                return [
                    b0[0], b1[0], b0[1], b1[1], b0[2], b1[2],
                    b0[3], b1[3], b0[4], b1[4], b0[5], b1[5],
                ]

            pending_tail = []
            accs_by_oc = {}

            def phase2(oc, p, es):
                if p == 0:
                    # the previous chunk's tail MUST be fully emitted
                    # before this chunk's first phase2 (which restarts the
                    # shared psum accumulator banks) so write-after-read
                    # ordering on those banks is correct
                    while pending_tail:
                        pending_tail.pop(0)()
                    accs_by_oc[oc] = [
                        accp.tile([P, 4 * P], f32, tag="acc", name=f"acc{oc}_{h}")
                        for h in range(2)
                    ]
                accs = accs_by_oc[oc]
                # 4 accumulators share each psum bank ("zero region"):
                # only the bank's first matmul starts the group (marking
                # the whole region pending-zero; siblings fresh-write),
                # and only its last one stops it.
                for qi in range(NQT):
                    a, ql = qi // 4, qi % 4
                    nc.tensor.matmul(
                        accs[a][:, ql * P : ql * P + M2],
                        es[:, qi * P : (qi + 1) * P].bitcast(bf16),
                        v_sb[:, p, :],
                        start=(p == 0 and ql == 0),
                        stop=(p == NKT - 1 and ql == 3),
                    )
                if p == NKT - 1:
                    pending_tail.extend(make_tail(oc, accs))

            # single fused stream over both q chunks: the PE pipeline never
            # drains at the chunk boundary
            inflight = []
            for k in range(NOC * NKT):
                oc, p = k // NKT, k % NKT
                for _ in range(min(3, len(pending_tail))):
                    pending_tail.pop(0)()
                st = stp.tile([P, QCH], f32, tag="st")
                for a in range(2):
                    nc.tensor.matmul(
                        st[:, a * HW : (a + 1) * HW],
                        vt[:, p * P : (p + 1) * P],
                        qt[:, oc * QCH + a * HW : oc * QCH + (a + 1) * HW],
                        start=True,
                        stop=True,
                    )
                # exp over the whole tile in one instruction; strict
                # ScalarE/DVE alternation so consecutive tiles never
                # queue behind each other on one engine
                es = esp.tile([P, QCH], i16, tag="es")
                if k % 2 == 0:
                    nc.scalar.activation(es[:].bitcast(bf16), st[:], EXP)
                else:
                    nc.vector.tensor_scalar(
                        es[:], st[:], SCH_A, SCH_B, MULT, ADD
                    )
                inflight.append((oc, p, es))
                if len(inflight) > LA:
                    phase2(*inflight.pop(0))
            for item in inflight:
                phase2(*item)
            while pending_tail:
                pending_tail.pop(0)()

    nc.compile()
    _CACHE["nc"] = nc
    return nc


def _in_maps(query, value):
    """Host-side sharding: slice per core into the layouts the kernel
    streams directly (transposes + bf16 V tiles with ones column)."""
    import ml_dtypes

    query = np.asarray(query, dtype=np.float32)
    value = np.asarray(value, dtype=np.float32)
    maps = []
    ones = np.ones((NKT, P, 1), np.float32)
    for c in range(NCORES):
        b, h = c // CORES_PER_B, c % CORES_PER_B
        qt = np.ascontiguousarray(query[b, h * RQ : (h + 1) * RQ, :].T)
        vt = np.ascontiguousarray(value[b].T)
        v3 = value[b].reshape(NKT, P, D)
        vs = np.ascontiguousarray(
            np.concatenate([v3, ones], axis=2)
            .transpose(1, 0, 2)
            .astype(ml_dtypes.bfloat16)
        )
        maps.append({"qt": qt, "vt": vt, "vs": vs})
    return maps


def run(query, value, trace=False):
    """Returns (output [4, 4096, 64] fp32, BassKernelResults)."""
    nc = _build()
    from concourse.bass_utils import run_bass_kernel_spmd

    res = run_bass_kernel_spmd(
        nc, _in_maps(query, value), core_ids=list(range(NCORES)), trace=trace
    )
    out = np.empty((B, SQ, D), np.float32)
    for c in range(NCORES):
        b, h = c // CORES_PER_B, c % CORES_PER_B
        out[b, h * RQ : (h + 1) * RQ, :] = res.results[c]["o"]
    return out, res


def kernel(query, value):
    out, _ = run(query, value)
    return out


# revision 24
# speedup vs baseline: 2.3199x; 1.0206x over previous
"""Trainium2 Bass kernel for nn_AttentionLayer_86629490360750.

reference:
    scores = einsum('bqd,bkd->bqk', query, value)   # no 1/sqrt(d) scaling
    dist   = softmax(scores, axis=-1)
    out    = einsum('bqk,bkd->bqd', dist, value)

Shapes: query/value [4, 4096, 64] fp32.

Sharding: 8 cores; core c handles batch b = c//2, query rows
[h*2048, (h+1)*2048) with h = c%2.  Each core sees all of value[b], so
there are no collectives.  Host-side layout per core:
  - qt [64, 2048]: Q^T slice (contraction dim on partitions),
  - vt [64, 4096]: V^T (phase-1 stationary tiles),
  - vs [128, 32, 65]: natural V tiles + ones column, in bf16 (the ones
    column makes the PV matmul accumulate the softmax denominator).

Per-core algorithm, pipelined in 128kv x 512q "half tiles" (no max
subtraction: scores are N(0, 64), so exp stays in fp32/bf16 range):
  phase 1  S^T half [128 kv, 512 q] = V^T.T @ Q^T   (PE, f32r, 1 bank)
  exp      es = exp(S^T) in bf16, spread over THREE engines:
             - ScalarE: exact exp activation (bf16 out)
             - DVE / GPSIMD: Schraudolph fast-exp -- one tensor_scalar
               int16(s*128/ln2 + b) whose bits read as bf16 give
               exp(s) to ~1.5% elementwise; softmax renormalization
               cancels most of it (measured ~5e-3 output rel err)
  phase 2  ctx[q 128, 65] += es^T(kv,q).T @ [V|1]   (PE, bf16): in the
             cost model a bf16 matmul charges out-free-size cycles
             regardless of contraction depth, so 65-wide outputs make
             this ~2x cheaper than the ctx^T orientation and need no
             transposes.  4 accumulators pack per psum bank; phase2
             trails phase1 by LA halves so the in-order PE queue always
             has independent work while exp is in flight.
  tail     one strided reciprocal per bank (4 denominators at once,
           DVE), scale muls spread over ScalarE/DVE/GPSIMD, DMA out.

PE is the bottleneck: phase 1 streams 65536 columns and phase 2 33280
per core, ~41us at 2.4GHz; exp (128 halves) is split ~48/44/36 across
ScalarE/DVE/GPSIMD (~29us each) and hides underneath along with DMA
and the tail.
"""

import math
import os
import sys

import numpy as np

for _TRN_REPO in ("/opt/trn_rl_repo", "/root/.axon_site/_ro/trn_rl_repo"):
    if os.path.isdir(_TRN_REPO):
        if _TRN_REPO not in sys.path:
            sys.path.insert(0, _TRN_REPO)
        break

B, SQ, SKV, D = 4, 4096, 4096, 64
NCORES = 8
CORES_PER_B = NCORES // B          # 2
RQ = SQ // CORES_PER_B             # 2048 query rows per core
P = 128
NKT = SKV // P                     # 32 kv tiles
QCH = 1024                         # q chunk (psum accumulator granularity)
NOC = RQ // QCH                    # 2
M2 = D + 1                         # 65: V plus ones column
NQT = QCH // P                     # 8 q sub-tiles per chunk
HW = 512                           # half-tile width (1 psum bank)
ES_BUFS = 6                        # es pool depth (sweepable)
ST_BUFS = 3                        # score psum tiles, two banks each
NWARM = 24                         # PE ramp warm matmuls (sweepable)
LA = 2                             # phase2 lookahead in kv tiles (sweepable)

# Schraudolph fast-exp: bits of int16(s*A + B) read as bf16 ~= exp(s).
SCH_A = 128.0 / math.log(2.0)
SCH_B = 127.0 * 128.0 - 3.15


_CACHE = {}


def _build():
    if "nc" in _CACHE:
        return _CACHE["nc"]

    import concourse.bass as bass  # noqa: F401
    import concourse.mybir as mybir
    import concourse.tile as tile
    from concourse import bacc

    f32 = mybir.dt.float32
    f32r = mybir.dt.float32r
    bf16 = mybir.dt.bfloat16
    i16 = mybir.dt.int16
    EXP = mybir.ActivationFunctionType.Exp
    MULT = mybir.AluOpType.mult
    ADD = mybir.AluOpType.add

    nc = bacc.Bacc(
        trn_type="TRN2",
        target_bir_lowering=False,
        debug=False,
        enable_asserts=False,
    )
    qt_d = nc.dram_tensor("qt", [D, RQ], f32, kind="ExternalInput").ap()
    vt_d = nc.dram_tensor("vt", [D, SKV], f32, kind="ExternalInput").ap()
    vs_d = nc.dram_tensor("vs", [P, NKT, M2], bf16, kind="ExternalInput").ap()
    o_d = nc.dram_tensor("o", [RQ, D], f32, kind="ExternalOutput").ap()

    with tile.TileContext(nc) as tc:
        with (
            tc.tile_pool(name="const", bufs=1) as const,
            tc.tile_pool(name="sb", bufs=1) as sb,
            tc.tile_pool(name="es", bufs=ES_BUFS) as esp,
            tc.tile_pool(name="outp", bufs=2) as outp,
            tc.tile_pool(name="rp", bufs=4) as rp,
            tc.tile_pool(name="st", bufs=ST_BUFS, space="PSUM") as stp,
            tc.tile_pool(name="acc", bufs=2, space="PSUM") as accp,
        ):
            # PE p-state warmup: tiny bf16 matmuls from t~0 keep the PE
            # ramp clock running while the input DMAs land.
            wz = const.tile([P, P], bf16)
            nc.vector.memset(wz[:], 0.0)
            warm = stp.tile([P, QCH], f32, tag="st")
            for w in range(NWARM):
                nc.tensor.matmul(
                    warm[:, (w % 4) * P : (w % 4 + 1) * P],
                    wz[:],
                    wz[:],
                    start=True,
                    stop=True,
                )

            qt = sb.tile([D, RQ], f32r)
            vt = sb.tile([D, SKV], f32r)
            v_sb = sb.tile([P, NKT, M2], bf16)

            # Input DMAs, ordered/chunked by first-use time: the whole
            # first-chunk qt, then vt in graduated pieces, vs between.
            nc.sync.dma_start(qt[:, 0:QCH], qt_d[:, 0:QCH].bitcast(f32r))
            nc.sync.dma_start(vt[:, 0:256], vt_d[:, 0:256].bitcast(f32r))
            nc.sync.dma_start(vt[:, 256:1024], vt_d[:, 256:1024].bitcast(f32r))
            nc.sync.dma_start(v_sb[:, 0:8, :], vs_d[:, 0:8, :])
            nc.sync.dma_start(vt[:, 1024:2048], vt_d[:, 1024:2048].bitcast(f32r))
            nc.sync.dma_start(v_sb[:, 8:NKT, :], vs_d[:, 8:NKT, :])
            nc.sync.dma_start(vt[:, 2048:SKV], vt_d[:, 2048:SKV].bitcast(f32r))
            nc.sync.dma_start(qt[:, QCH:RQ], qt_d[:, QCH:RQ].bitcast(f32r))

            def make_tail(oc, accs):
                """Per-bank normalize: one strided reciprocal covering the
                bank's 4 denominator columns (DVE), then 4 scale muls on a
                per-bank engine (ScalarE for bank 0, DVE for bank 1 -- one
                output tile per bank so there are no cross-engine same-tile
                write-order serializations), then the bank's output DMA.
                Returned as emission closures interleaved into the next
                chunk's loop."""
                final = oc == NOC - 1
                banks = []
                for a in range(2):
                    acc = accs[a]
                    if not final:
                        # non-final chunk: evacuate the psum bank to SBUF
                        # first (one copy per engine, in parallel) so the
                        # next chunk's accumulation isn't gated on the
                        # whole normalize+DMA chain
                        sbacc = sb.tile(
                            [P, 4 * P], f32, name=f"sbacc{oc}_{a}"
                        )
                    src_t = acc if final else sbacc
                    acc3 = src_t.rearrange("p (s c) -> p s c", c=P)
                    r4 = rp.tile([P, 4], f32, tag=f"r4_{a}", name=f"r4_{oc}_{a}")
                    ot = outp.tile(
                        [P, 4, D], f32, tag=f"ot{a}", name=f"ot{oc}_{a}"
                    )

                    def copy(a=a, acc=acc, sbacc=None if final else sbacc):
                        if a == 0:
                            nc.scalar.copy(sbacc[:], acc[:])
                        else:
                            nc.vector.tensor_copy(sbacc[:], acc[:])

                    def recip(acc3=acc3, r4=r4):
                        nc.vector.reciprocal(r4[:], acc3[:, :, D : D + 1])

                    def mul(ql, a=a, src_t=src_t, r4=r4, ot=ot):
                        def go():
                            dst = ot[:, ql, :]
                            src = src_t[:, ql * P : ql * P + D]
                            r = r4[:, ql : ql + 1]
                            if a == 0:
                                nc.scalar.mul(dst, src, r)
                            else:
                                nc.vector.tensor_scalar_mul(dst, src, r)

                        return go

                    def dma(a=a, ot=ot):
                        row0 = oc * QCH + a * HW
                        nc.sync.dma_start(
                            o_d[row0 : row0 + HW, :].rearrange(
                                "(t p) d -> p t d", p=P
                            ),
                            ot[:],
                        )

                    pieces = [] if final else [copy]
                    pieces += [recip, mul(0), mul(1), mul(2), mul(3), dma]
                    banks.append(pieces)
                b0, b1 = banks
                return [x for pair in zip(b0, b1) for x in pair]

            pending_tail = []
            accs_by_oc = {}

            def phase2(oc, p, es):
                if p == 0:
                    # the previous chunk's tail MUST be fully emitted
                    # before this chunk's first phase2 (which restarts the
                    # shared psum accumulator banks) so write-after-read
                    # ordering on those banks is correct
                    while pending_tail:
                        pending_tail.pop(0)()
                    accs_by_oc[oc] = [
                        accp.tile([P, 4 * P], f32, tag="acc", name=f"acc{oc}_{h}")
                        for h in range(2)
                    ]
                accs = accs_by_oc[oc]
                # 4 accumulators share each psum bank ("zero region"):
                # only the bank's first matmul starts the group (marking
                # the whole region pending-zero; siblings fresh-write),
                # and only its last one stops it.
                for qi in range(NQT):
                    a, ql = qi // 4, qi % 4
                    nc.tensor.matmul(
                        accs[a][:, ql * P : ql * P + M2],
                        es[:, qi * P : (qi + 1) * P].bitcast(bf16),
                        v_sb[:, p, :],
                        start=(p == 0 and ql == 0),
                        stop=(p == NKT - 1 and ql == 3),
                    )
                if p == NKT - 1:
                    pending_tail.extend(make_tail(oc, accs))

            # single fused stream over both q chunks: the PE pipeline never
            # drains at the chunk boundary
            inflight = []
            for k in range(NOC * NKT):
                oc, p = k // NKT, k % NKT
                for _ in range(min(3, len(pending_tail))):
                    pending_tail.pop(0)()
                st = stp.tile([P, QCH], f32, tag="st")
                for a in range(2):
                    nc.tensor.matmul(
                        st[:, a * HW : (a + 1) * HW],
                        vt[:, p * P : (p + 1) * P],
                        qt[:, oc * QCH + a * HW : oc * QCH + (a + 1) * HW],
                        start=True,
                        stop=True,
                    )
                # exp over the whole tile in one instruction; strict
                # ScalarE/DVE alternation so consecutive tiles never
                # queue behind each other on one engine.  The last two
                # tiles split halves across BOTH engines so the pipeline
                # drains ~2x faster into the kernel tail.
                es = esp.tile([P, QCH], i16, tag="es")
                if k >= NOC * NKT - 2:
                    nc.scalar.activation(
                        es[:, 0:HW].bitcast(bf16), st[:, 0:HW], EXP
                    )
                    nc.vector.tensor_scalar(
                        es[:, HW:QCH], st[:, HW:QCH], SCH_A, SCH_B, MULT, ADD
                    )
                elif k % 2 == 0:
                    nc.scalar.activation(es[:].bitcast(bf16), st[:], EXP)
                else:
                    nc.vector.tensor_scalar(
                        es[:], st[:], SCH_A, SCH_B, MULT, ADD
                    )
                inflight.append((oc, p, es))
                if len(inflight) > LA:
                    phase2(*inflight.pop(0))
            for item in inflight:
                phase2(*item)
            while pending_tail:
                pending_tail.pop(0)()

    nc.compile()
    _CACHE["nc"] = nc
    return nc


def _in_maps(query, value):
    """Host-side sharding: slice per core into the layouts the kernel
    streams directly (transposes + bf16 V tiles with ones column)."""
    import ml_dtypes

    query = np.asarray(query, dtype=np.float32)
    value = np.asarray(value, dtype=np.float32)
    maps = []
    ones = np.ones((NKT, P, 1), np.float32)
    for c in range(NCORES):
        b, h = c // CORES_PER_B, c % CORES_PER_B
        qt = np.ascontiguousarray(query[b, h * RQ : (h + 1) * RQ, :].T)
        vt = np.ascontiguousarray(value[b].T)
        v3 = value[b].reshape(NKT, P, D)
        vs = np.ascontiguousarray(
            np.concatenate([v3, ones], axis=2)
            .transpose(1, 0, 2)
            .astype(ml_dtypes.bfloat16)
        )
        maps.append({"qt": qt, "vt": vt, "vs": vs})
    return maps


def run(query, value, trace=False):
    """Returns (output [4, 4096, 64] fp32, BassKernelResults)."""
    nc = _build()
    from concourse.bass_utils import run_bass_kernel_spmd

    res = run_bass_kernel_spmd(
        nc, _in_maps(query, value), core_ids=list(range(NCORES)), trace=trace
    )
    out = np.empty((B, SQ, D), np.float32)
    for c in range(NCORES):
        b, h = c // CORES_PER_B, c % CORES_PER_B
        out[b, h * RQ : (h + 1) * RQ, :] = res.results[c]["o"]
    return out, res


def kernel(query, value):
    out, _ = run(query, value)
    return out
